# revision 7
# baseline (speedup 1.0000x reference)
# Causal self-attention (B=4, T=2048, C=1024, 16 heads) on 8 NeuronCores.
# Sharding: core = (batch b = core//2) x (head-group hg = core%2, 8 heads each).
# Each core computes its 8 heads' attention for its batch plus the row-slice of
# the output projection; the host sums the two partial projections per batch.
#
# Self-contained: hardcodes shapes; builds + compiles the Bass program once.

import numpy as np
import ml_dtypes

import concourse.bass as bass
import concourse.tile as tile
from concourse import bacc, mybir
from concourse.bass import AP
from concourse.bass_utils import run_bass_kernel_spmd

F32 = mybir.dt.float32
BF16 = mybir.dt.bfloat16

B, T, C = 4, 2048, 1024
NH, HS = 16, 64
NHPC = 8          # heads per core
D = NHPC * HS     # 512: per-core qkv width
NCORES = 8
TT = T // 128     # 16 token tiles
TC = T // 512     # 4 token chunks
CT = C // 128     # 8 contraction tiles
DT = D // 128     # 4 d-tiles of qT/kT
VW = 68           # per-head stride in v tile: [v(64) | ones | pad3]

_cache = {}


def _bcast_row(ap, nrep=128):
    """AP that reads a [1, N] slice nrep times (free-dim step-0 broadcast)."""
    return AP(ap.tensor, ap.offset, [ap.ap[0], [0, nrep]] + ap.ap[1:])


def _build():
    nc = bacc.Bacc("TRN2", target_bir_lowering=False, debug=False,
                   num_devices=NCORES)

    xT = nc.dram_tensor("xT", [C, T], BF16, kind="ExternalInput")
    wq = nc.dram_tensor("wq", [C, D], BF16, kind="ExternalInput")
    wk = nc.dram_tensor("wk", [C, D], BF16, kind="ExternalInput")
    wv = nc.dram_tensor("wv", [C, D], BF16, kind="ExternalInput")
    wp = nc.dram_tensor("wp", [D, C], BF16, kind="ExternalInput")
    bq = nc.dram_tensor("bq", [128, DT], F32, kind="ExternalInput")
    bk = nc.dram_tensor("bk", [128, DT], F32, kind="ExternalInput")
    bvbc = nc.dram_tensor("bvbc", [128, D], F32, kind="ExternalInput")
    ypT = nc.dram_tensor("ypT", [C, T], F32, kind="ExternalOutput")

    with tile.TileContext(nc) as tc:
        import contextlib
        ctx = contextlib.ExitStack()
        with ctx:
            cpool = ctx.enter_context(tc.tile_pool(name="consts", bufs=1))
            xpool = ctx.enter_context(tc.tile_pool(name="x", bufs=1))
            wpool = ctx.enter_context(tc.tile_pool(name="w", bufs=1))
            qkpool = ctx.enter_context(tc.tile_pool(name="qk", bufs=1))
            vpool = ctx.enter_context(tc.tile_pool(name="v", bufs=1))
            opool = ctx.enter_context(tc.tile_pool(name="oT", bufs=1))
            ptpool = ctx.enter_context(tc.tile_pool(name="pt", bufs=8))
            ypool = ctx.enter_context(tc.tile_pool(name="y", bufs=4))
            avpool = ctx.enter_context(tc.tile_pool(name="av", bufs=3))
            rpool = ctx.enter_context(tc.tile_pool(name="rbc", bufs=3))
            ripool = ctx.enter_context(tc.tile_pool(name="ri", bufs=3))
            qkv_ps = ctx.enter_context(
                tc.tile_pool(name="qkv_ps", bufs=2, space="PSUM"))
            s_ps = ctx.enter_context(
                tc.tile_pool(name="s_ps", bufs=2, space="PSUM"))
            o_ps = ctx.enter_context(
                tc.tile_pool(name="o_ps", bufs=2, space="PSUM"))

            # ---- input DMAs (ordered by first use: wv, xT by chunk, wq/wk) ----
            wvt = [wpool.tile([128, D], BF16, tag=f"wv{i}", name=f"wv{i}") for i in range(CT)]
            for i in range(CT):
                eng = nc.sync if i % 2 == 0 else nc.scalar
                eng.dma_start(wvt[i][:], wv.ap()[i * 128:(i + 1) * 128, :])
            xt = [xpool.tile([128, T], BF16, tag=f"xT{i}", name=f"xT{i}") for i in range(CT)]
            for cch in range(TC):
                for i in range(CT):
                    eng = nc.sync if i % 2 == 0 else nc.scalar
                    eng.dma_start(
                        xt[i][:, cch * 512:(cch + 1) * 512],
                        xT.ap()[i * 128:(i + 1) * 128, cch * 512:(cch + 1) * 512])
            wqt = [wpool.tile([128, D], BF16, tag=f"wq{i}", name=f"wq{i}") for i in range(CT)]
            wkt = [wpool.tile([128, D], BF16, tag=f"wk{i}", name=f"wk{i}") for i in range(CT)]
            for i in range(CT):
                nc.sync.dma_start(wqt[i][:], wq.ap()[i * 128:(i + 1) * 128, :])
                nc.scalar.dma_start(wkt[i][:], wk.ap()[i * 128:(i + 1) * 128, :])
            wpt = [wpool.tile([128, C], BF16, tag=f"wp{i}", name=f"wp{i}") for i in range(DT)]
            for i in range(DT):
                nc.sync.dma_start(wpt[i][:], wp.ap()[i * 128:(i + 1) * 128, :])
            bqt = cpool.tile([128, DT], F32, tag="bq")
            bkt = cpool.tile([128, DT], F32, tag="bk")
            bvt = cpool.tile([128, D], F32, tag="bv")
            nc.sync.dma_start(bqt[:], bq.ap())
            nc.sync.dma_start(bkt[:], bk.ap())
            nc.sync.dma_start(bvt[:], bvbc.ap())

            ones8 = cpool.tile([128, NHPC], F32, tag="ones8")
            nc.vector.memset(ones8[:], 1.0)
            ones_t = cpool.tile([128, 512], F32, tag="ones_t")
            nc.vector.memset(ones_t[:], 1.0)
            masks = []
            for t in range(4):
                mf = cpool.tile([128, 512], F32, tag=f"mf{t}", name=f"mf{t}")
                nc.gpsimd.affine_select(
                    mf[:], ones_t[:], pattern=[[1, 512]],
                    compare_op=mybir.AluOpType.is_ge, fill=0.0,
                    base=-(128 * t), channel_multiplier=-1)
                mb = cpool.tile([128, 512], BF16, tag=f"mb{t}", name=f"mb{t}")
                nc.vector.tensor_copy(mb[:], mf[:])
                masks.append(mb)

            # ---- v = x @ Wv + bv, token-major, [v|ones] per head ----
            vt = [vpool.tile([128, NHPC * VW], BF16, tag=f"v{j}", name=f"v{j}")
                  for j in range(TT)]
            for j in range(TT):
                ps = qkv_ps.tile([128, D], F32, tag="qkv", name="qkvps")
                for ct in range(CT):
                    nc.tensor.matmul(
                        ps[:], xt[ct][:, j * 128:(j + 1) * 128], wvt[ct][:],
                        start=(ct == 0), stop=(ct == CT - 1))
                vre = vt[j][:].rearrange("p (h x) -> p h x", h=NHPC)
                nc.vector.tensor_copy(
                    vre[:, :, 64:65],
                    ones8[:].rearrange("p (h x) -> p h x", x=1))
                nc.vector.tensor_add(
                    vre[:, :, 0:64],
                    ps[:].rearrange("p (h x) -> p h x", h=NHPC),
                    bvt[:].rearrange("p (h x) -> p h x", h=NHPC))

            # ---- qT/kT per d-tile + attention per head pair ----
            qT = [qkpool.tile([128, T], BF16, tag=f"q{d}", name=f"q{d}") for d in range(DT)]
            kT = [qkpool.tile([128, T], BF16, tag=f"k{d}", name=f"k{d}") for d in range(DT)]
            yt = {}
            # ---- output projection (emitted per chunk at hp==3) ----
            def emit_proj(c):
                for o in range(CT):
                    ps = qkv_ps.tile([128, 512], F32, tag="qkv", name="qkvps")
                    for hp in range(DT):
                        nc.tensor.matmul(
                            ps[:],
                            wpt[hp][:, o * 128:(o + 1) * 128],
                            oT_tile(opool, yt, hp)[:, c * 512:(c + 1) * 512],
                            start=(hp == 0), stop=(hp == DT - 1))
                    ys = ypool.tile([128, 512], F32, tag="y", name="ys")
                    nc.vector.tensor_copy(ys[:], ps[:])
                    nc.gpsimd.dma_start(
                        ypT.ap()[o * 128:(o + 1) * 128,
                                 c * 512:(c + 1) * 512], ys[:])



            def emit_qk_group(hp, idx):
                # idx 0..7: (chunk, q|k)
                c, which = idx // 2, idx % 2
                wt_, bt_, out = ((wqt, bqt, qT), (wkt, bkt, kT))[which]
                ps = qkv_ps.tile([128, 512], F32, tag="qkv", name="qkvps")
                for ct in range(CT):
                    nc.tensor.matmul(
                        ps[:],
                        wt_[ct][:, hp * 128:(hp + 1) * 128],
                        xt[ct][:, c * 512:(c + 1) * 512],
                        start=(ct == 0), stop=(ct == CT - 1))
                nc.vector.tensor_scalar_add(
                    out[hp][:, c * 512:(c + 1) * 512], ps[:],
                    bt_[:, hp:hp + 1])

            for hp in range(DT):
                if hp == 0:
                    for idx in range(8):
                        emit_qk_group(0, idx)

                # attention for heads (2*hp, 2*hp+1); qk of hp+1 interleaved
                for c in range(TC):
                    if hp + 1 < DT:
                        emit_qk_group(hp + 1, 2 * c)
                        emit_qk_group(hp + 1, 2 * c + 1)
                    njt = 4 * c + 4
                    op0 = o_ps.tile([128, 512], F32, tag="o", name="ops")
                    op1 = o_ps.tile([128, 512], F32, tag="o", name="ops")
                    for jt in range(njt):
                        sp = s_ps.tile([128, 1024], F32, tag="s", name="sps")
                        for half in range(2):
                            nc.tensor.matmul(
                                sp[:, half * 512:(half + 1) * 512],
                                kT[hp][half * 64:(half + 1) * 64,
                                       jt * 128:(jt + 1) * 128],
                                qT[hp][half * 64:(half + 1) * 64,
                                       c * 512:(c + 1) * 512],
                                start=True, stop=True)
                        pt = ptpool.tile([128, 1024], BF16, tag="pt", name="pt")
                        nc.scalar.activation(
                            pt[:], sp[:],
                            mybir.ActivationFunctionType.Exp, scale=0.125)
                        if jt >= 4 * c:
                            t = jt - 4 * c
                            nc.vector.tensor_mul(
                                pt[:, 0:512], pt[:, 0:512], masks[t][:])
                            pv = pt[:, 512:1024]
                            nc.gpsimd.affine_select(
                                pv, pv, pattern=[[1, 512]],
                                compare_op=mybir.AluOpType.is_ge, fill=0.0,
                                base=-(128 * t), channel_multiplier=-1)
                        for half, op in ((0, op0), (1, op1)):
                            h = 2 * hp + half
                            nc.tensor.matmul(
                                op[0:65, :],
                                vt[jt][:, h * VW:h * VW + 65],
                                pt[:, half * 512:(half + 1) * 512],
                                start=(jt == 0), stop=(jt == njt - 1))
                    # normalize: rows 0:64 / row 64 (rowsum); halves interleaved
                    ri0 = ripool.tile([128, 512], F32, tag="ri", name="ri")
                    ri1 = ripool.tile([128, 512], F32, tag="ri", name="ri")
                    nc.vector.tensor_copy(ri0[64:65, :], op0[64:65, :])
                    nc.vector.tensor_copy(ri1[64:65, :], op1[64:65, :])
                    rb0 = rpool.tile([128, 512], F32, tag="rb", name="rb")
                    rb1 = rpool.tile([128, 512], F32, tag="rb", name="rb")
                    nc.sync.dma_start(
                        rb0[0:64, :], _bcast_row(ri0[64:65, :], nrep=64))
                    nc.sync.dma_start(
                        rb1[0:64, :], _bcast_row(ri1[64:65, :], nrep=64))
                    nc.vector.reciprocal_approx_fast(rb0[0:64, :], rb0[0:64, :])
                    nc.vector.reciprocal_approx_fast(rb1[0:64, :], rb1[0:64, :])
                    nc.vector.tensor_mul(
                        oT_tile(opool, yt, hp)[0:64, c * 512:(c + 1) * 512],
                        op0[0:64, :], rb0[0:64, :])
                    av = avpool.tile([128, 512], BF16, tag="av", name="av")
                    nc.vector.tensor_mul(
                        av[0:64, :], op1[0:64, :], rb1[0:64, :])
                    nc.sync.dma_start(
                        oT_tile(opool, yt, hp)[64:128, c * 512:(c + 1) * 512],
                        av[0:64, :])
                    if hp == DT - 1:
                        emit_proj(c)

    nc.compile()
    return nc


def oT_tile(opool, yt, hp):
    if hp not in yt:
        yt[hp] = opool.tile([128, T], BF16, tag=f"oT{hp}", name=f"oT{hp}")
    return yt[hp]


def _shard_inputs(x, Wk, bk, Wq, bq, Wv, bv, Wp, bp):
    bf = ml_dtypes.bfloat16
    in_maps = []
    for core in range(NCORES):
        b, hg = core // 2, core % 2
        sl = slice(hg * D, (hg + 1) * D)
        in_maps.append({
            "xT": np.ascontiguousarray(x[b].T).astype(bf),
            "wq": np.ascontiguousarray(Wq[:, sl]).astype(bf),
            "wk": np.ascontiguousarray(Wk[:, sl]).astype(bf),
            "wv": np.ascontiguousarray(Wv[:, sl]).astype(bf),
            "wp": np.ascontiguousarray(Wp[sl, :]).astype(bf),
            "bq": np.ascontiguousarray(
                bq[sl].reshape(DT, 128).T).astype(np.float32),
            "bk": np.ascontiguousarray(
                bk[sl].reshape(DT, 128).T).astype(np.float32),
            "bvbc": np.ascontiguousarray(
                np.broadcast_to(bv[sl], (128, D))).astype(np.float32),
        })
    return in_maps


def kernel(x, Wk, bk, Wq, bq, Wv, bv, Wp, bp, _trace=False, _trace_kwargs=None):
    x, Wk, bk, Wq, bq, Wv, bv, Wp, bp = [
        np.asarray(a) for a in (x, Wk, bk, Wq, bq, Wv, bv, Wp, bp)]
    if "nc" not in _cache:
        _cache["nc"] = _build()
    nc = _cache["nc"]
    in_maps = _shard_inputs(x, Wk, bk, Wq, bq, Wv, bv, Wp, bp)
    kw = dict(_trace_kwargs or {})
    res = run_bass_kernel_spmd(nc, in_maps, core_ids=list(range(NCORES)),
                               trace=_trace, **kw)
    out = np.empty((B, T, C), np.float32)
    for b in range(B):
        yp = res.results[2 * b]["ypT"] + res.results[2 * b + 1]["ypT"]
        out[b] = yp.T + bp[None, :]
    if _trace:
        _cache["last_results"] = res
    return out


# revision 9
# speedup vs baseline: 1.1602x; 1.1602x over previous
# Causal self-attention (B=4, T=2048, C=1024, 16 heads) on 8 NeuronCores.
# Sharding: core = (batch b = core//2) x (head-group hg = core%2, 8 heads each).
# Each core computes its 8 heads' attention for its batch plus the row-slice of
# the output projection; the host sums the two partial projections per batch.
#
# Self-contained: hardcodes shapes; builds + compiles the Bass program once.

import numpy as np
import ml_dtypes

import concourse.bass as bass
import concourse.tile as tile
from concourse import bacc, mybir
from concourse.bass import AP
from concourse.bass_utils import run_bass_kernel_spmd

F32 = mybir.dt.float32
BF16 = mybir.dt.bfloat16

B, T, C = 4, 2048, 1024
NH, HS = 16, 64
NHPC = 8          # heads per core
D = NHPC * HS     # 512: per-core qkv width
NCORES = 8
TT = T // 128     # 16 token tiles
TC = T // 512     # 4 token chunks
CT = C // 128     # 8 contraction tiles
DT = D // 128     # 4 d-tiles of qT/kT
VW = 68           # per-head stride in v tile: [v(64) | ones | pad3]

_cache = {}


def _bcast_row(ap, nrep=128):
    """AP that reads a [1, N] slice nrep times (free-dim step-0 broadcast)."""
    return AP(ap.tensor, ap.offset, [ap.ap[0], [0, nrep]] + ap.ap[1:])


def _build():
    nc = bacc.Bacc("TRN2", target_bir_lowering=False, debug=False,
                   num_devices=NCORES)

    xT = nc.dram_tensor("xT", [C, T], BF16, kind="ExternalInput")
    wq = nc.dram_tensor("wq", [C, D], BF16, kind="ExternalInput")
    wk = nc.dram_tensor("wk", [C, D], BF16, kind="ExternalInput")
    wv = nc.dram_tensor("wv", [C, D], BF16, kind="ExternalInput")
    wp = nc.dram_tensor("wp", [D, C], BF16, kind="ExternalInput")
    bq = nc.dram_tensor("bq", [128, DT], F32, kind="ExternalInput")
    bk = nc.dram_tensor("bk", [128, DT], F32, kind="ExternalInput")
    bvbc = nc.dram_tensor("bvbc", [128, D], F32, kind="ExternalInput")
    ypT = nc.dram_tensor("ypT", [C, T], F32, kind="ExternalOutput")

    with tile.TileContext(nc) as tc:
        import contextlib
        ctx = contextlib.ExitStack()
        with ctx:
            cpool = ctx.enter_context(tc.tile_pool(name="consts", bufs=1))
            xpool = ctx.enter_context(tc.tile_pool(name="x", bufs=1))
            wpool = ctx.enter_context(tc.tile_pool(name="w", bufs=1))
            qkpool = ctx.enter_context(tc.tile_pool(name="qk", bufs=1))
            vpool = ctx.enter_context(tc.tile_pool(name="v", bufs=1))
            opool = ctx.enter_context(tc.tile_pool(name="oT", bufs=1))
            ptpool = ctx.enter_context(tc.tile_pool(name="pt", bufs=8))
            ypool = ctx.enter_context(tc.tile_pool(name="y", bufs=4))
            avpool = ctx.enter_context(tc.tile_pool(name="av", bufs=3))
            rpool = ctx.enter_context(tc.tile_pool(name="rbc", bufs=3))
            ripool = ctx.enter_context(tc.tile_pool(name="ri", bufs=3))
            qkv_ps = ctx.enter_context(
                tc.tile_pool(name="qkv_ps", bufs=2, space="PSUM"))
            s_ps = ctx.enter_context(
                tc.tile_pool(name="s_ps", bufs=2, space="PSUM"))
            o_ps = ctx.enter_context(
                tc.tile_pool(name="o_ps", bufs=2, space="PSUM"))

            # ---- input DMAs (ordered by first use: wv, xT by chunk, wq/wk) ----
            wvt = [wpool.tile([128, D], BF16, tag=f"wv{i}", name=f"wv{i}") for i in range(CT)]
            for i in range(CT):
                eng = nc.sync if i % 2 == 0 else nc.scalar
                eng.dma_start(wvt[i][:], wv.ap()[i * 128:(i + 1) * 128, :])
            xt = [xpool.tile([128, T], BF16, tag=f"xT{i}", name=f"xT{i}") for i in range(CT)]
            for cch in range(TC):
                for i in range(CT):
                    eng = nc.sync if i % 2 == 0 else nc.scalar
                    eng.dma_start(
                        xt[i][:, cch * 512:(cch + 1) * 512],
                        xT.ap()[i * 128:(i + 1) * 128, cch * 512:(cch + 1) * 512])
            wqt = [wpool.tile([128, D], BF16, tag=f"wq{i}", name=f"wq{i}") for i in range(CT)]
            wkt = [wpool.tile([128, D], BF16, tag=f"wk{i}", name=f"wk{i}") for i in range(CT)]
            for i in range(CT):
                nc.sync.dma_start(wqt[i][:], wq.ap()[i * 128:(i + 1) * 128, :])
                nc.scalar.dma_start(wkt[i][:], wk.ap()[i * 128:(i + 1) * 128, :])
            wpt = [wpool.tile([128, C], BF16, tag=f"wp{i}", name=f"wp{i}") for i in range(DT)]
            for i in range(DT):
                nc.sync.dma_start(wpt[i][:], wp.ap()[i * 128:(i + 1) * 128, :])
            bqt = cpool.tile([128, DT], F32, tag="bq")
            bkt = cpool.tile([128, DT], F32, tag="bk")
            bvt = cpool.tile([128, D], F32, tag="bv")
            nc.sync.dma_start(bqt[:], bq.ap())
            nc.sync.dma_start(bkt[:], bk.ap())
            nc.sync.dma_start(bvt[:], bvbc.ap())

            ones8 = cpool.tile([128, NHPC], F32, tag="ones8")
            nc.vector.memset(ones8[:], 1.0)
            ones_t = cpool.tile([128, 512], F32, tag="ones_t")
            nc.vector.memset(ones_t[:], 1.0)
            masks = []
            for t in range(4):
                mf = cpool.tile([128, 512], F32, tag=f"mf{t}", name=f"mf{t}")
                nc.gpsimd.affine_select(
                    mf[:], ones_t[:], pattern=[[1, 512]],
                    compare_op=mybir.AluOpType.is_ge, fill=0.0,
                    base=-(128 * t), channel_multiplier=-1)
                mb = cpool.tile([128, 512], BF16, tag=f"mb{t}", name=f"mb{t}")
                nc.vector.tensor_copy(mb[:], mf[:])
                masks.append(mb)

            # ---- v = x @ Wv + bv, token-major, [v|ones] per head ----
            vt = [vpool.tile([128, NHPC * VW], BF16, tag=f"v{j}", name=f"v{j}")
                  for j in range(TT)]
            for j in range(TT):
                ps = qkv_ps.tile([128, D], F32, tag="qkv", name="qkvps")
                for ct in range(CT):
                    nc.tensor.matmul(
                        ps[:], xt[ct][:, j * 128:(j + 1) * 128], wvt[ct][:],
                        start=(ct == 0), stop=(ct == CT - 1))
                vre = vt[j][:].rearrange("p (h x) -> p h x", h=NHPC)
                nc.vector.tensor_copy(
                    vre[:, :, 64:65],
                    ones8[:].rearrange("p (h x) -> p h x", x=1))
                nc.vector.tensor_add(
                    vre[:, :, 0:64],
                    ps[:].rearrange("p (h x) -> p h x", h=NHPC),
                    bvt[:].rearrange("p (h x) -> p h x", h=NHPC))

            # ---- qT/kT per d-tile + attention per head pair ----
            qT = [qkpool.tile([128, T], BF16, tag=f"q{d}", name=f"q{d}") for d in range(DT)]
            kT = [qkpool.tile([128, T], BF16, tag=f"k{d}", name=f"k{d}") for d in range(DT)]
            yt = {}
            # ---- output projection (emitted per chunk at hp==3) ----
            def emit_proj(c):
                for o in range(CT):
                    ps = qkv_ps.tile([128, 512], F32, tag="qkv", name="qkvps")
                    for hp in range(DT):
                        nc.tensor.matmul(
                            ps[:],
                            wpt[hp][:, o * 128:(o + 1) * 128],
                            oT_tile(opool, yt, hp)[:, c * 512:(c + 1) * 512],
                            start=(hp == 0), stop=(hp == DT - 1))
                    ys = ypool.tile([128, 512], F32, tag="y", name="ys")
                    nc.vector.tensor_copy(ys[:], ps[:])
                    nc.sync.dma_start(
                        ypT.ap()[o * 128:(o + 1) * 128,
                                 c * 512:(c + 1) * 512], ys[:])



            def emit_qk_group(hp, idx):
                # idx 0..7: (chunk, q|k)
                c, which = idx // 2, idx % 2
                wt_, bt_, out = ((wqt, bqt, qT), (wkt, bkt, kT))[which]
                ps = qkv_ps.tile([128, 512], F32, tag="qkv", name="qkvps")
                for ct in range(CT):
                    nc.tensor.matmul(
                        ps[:],
                        wt_[ct][:, hp * 128:(hp + 1) * 128],
                        xt[ct][:, c * 512:(c + 1) * 512],
                        start=(ct == 0), stop=(ct == CT - 1))
                nc.scalar.activation(
                    out[hp][:, c * 512:(c + 1) * 512], ps[:],
                    mybir.ActivationFunctionType.Identity,
                    bias=bt_[:, hp:hp + 1])

            for hp in range(DT):
                if hp == 0:
                    for idx in range(8):
                        emit_qk_group(0, idx)

                # attention for heads (2*hp, 2*hp+1); qk of hp+1 interleaved
                for c in range(TC):
                    if hp + 1 < DT:
                        emit_qk_group(hp + 1, 2 * c)
                        emit_qk_group(hp + 1, 2 * c + 1)
                    njt = 4 * c + 4
                    op0 = o_ps.tile([128, 512], F32, tag="o", name="ops")
                    op1 = o_ps.tile([128, 512], F32, tag="o", name="ops")
                    for jt in range(njt):
                        sp = s_ps.tile([128, 1024], F32, tag="s", name="sps")
                        for half in range(2):
                            nc.tensor.matmul(
                                sp[:, half * 512:(half + 1) * 512],
                                kT[hp][half * 64:(half + 1) * 64,
                                       jt * 128:(jt + 1) * 128],
                                qT[hp][half * 64:(half + 1) * 64,
                                       c * 512:(c + 1) * 512],
                                start=True, stop=True)
                        pt = ptpool.tile([128, 1024], BF16, tag="pt", name="pt")
                        nc.scalar.activation(
                            pt[:], sp[:],
                            mybir.ActivationFunctionType.Exp, scale=0.125)
                        if jt >= 4 * c:
                            t = jt - 4 * c
                            nc.vector.tensor_mul(
                                pt[:, 0:512], pt[:, 0:512], masks[t][:])
                            pv = pt[:, 512:1024]
                            nc.gpsimd.affine_select(
                                pv, pv, pattern=[[1, 512]],
                                compare_op=mybir.AluOpType.is_ge, fill=0.0,
                                base=-(128 * t), channel_multiplier=-1)
                        for half, op in ((0, op0), (1, op1)):
                            h = 2 * hp + half
                            nc.tensor.matmul(
                                op[0:65, :],
                                vt[jt][:, h * VW:h * VW + 65],
                                pt[:, half * 512:(half + 1) * 512],
                                start=(jt == 0), stop=(jt == njt - 1))
                    # normalize: copy psum out fast (frees o_ps), then
                    # bcast rowsum, reciprocal, scale — all off-psum
                    tm0 = ripool.tile([128, 512], F32, tag="ri", name="ri")
                    tm1 = ripool.tile([128, 512], F32, tag="ri", name="ri")
                    nc.vector.tensor_copy(tm0[0:65, :], op0[0:65, :])
                    nc.vector.tensor_copy(tm1[0:65, :], op1[0:65, :])
                    rb0 = rpool.tile([128, 512], F32, tag="rb", name="rb")
                    rb1 = rpool.tile([128, 512], F32, tag="rb", name="rb")
                    nc.sync.dma_start(
                        rb0[0:64, :], _bcast_row(tm0[64:65, :], nrep=64))
                    nc.sync.dma_start(
                        rb1[0:64, :], _bcast_row(tm1[64:65, :], nrep=64))
                    nc.vector.reciprocal_approx_fast(rb0[0:64, :], rb0[0:64, :])
                    nc.vector.reciprocal_approx_fast(rb1[0:64, :], rb1[0:64, :])
                    nc.vector.tensor_mul(
                        oT_tile(opool, yt, hp)[0:64, c * 512:(c + 1) * 512],
                        tm0[0:64, :], rb0[0:64, :])
                    av = avpool.tile([128, 512], BF16, tag="av", name="av")
                    nc.vector.tensor_mul(
                        av[0:64, :], tm1[0:64, :], rb1[0:64, :])
                    nc.sync.dma_start(
                        oT_tile(opool, yt, hp)[64:128, c * 512:(c + 1) * 512],
                        av[0:64, :])
                    if hp == DT - 1:
                        emit_proj(c)

    nc.compile()
    return nc


def oT_tile(opool, yt, hp):
    if hp not in yt:
        yt[hp] = opool.tile([128, T], BF16, tag=f"oT{hp}", name=f"oT{hp}")
    return yt[hp]


def _shard_inputs(x, Wk, bk, Wq, bq, Wv, bv, Wp, bp):
    bf = ml_dtypes.bfloat16
    in_maps = []
    for core in range(NCORES):
        b, hg = core // 2, core % 2
        sl = slice(hg * D, (hg + 1) * D)
        in_maps.append({
            "xT": np.ascontiguousarray(x[b].T).astype(bf),
            "wq": np.ascontiguousarray(Wq[:, sl]).astype(bf),
            "wk": np.ascontiguousarray(Wk[:, sl]).astype(bf),
            "wv": np.ascontiguousarray(Wv[:, sl]).astype(bf),
            "wp": np.ascontiguousarray(Wp[sl, :]).astype(bf),
            "bq": np.ascontiguousarray(
                bq[sl].reshape(DT, 128).T).astype(np.float32),
            "bk": np.ascontiguousarray(
                bk[sl].reshape(DT, 128).T).astype(np.float32),
            "bvbc": np.ascontiguousarray(
                np.broadcast_to(bv[sl], (128, D))).astype(np.float32),
        })
    return in_maps


def kernel(x, Wk, bk, Wq, bq, Wv, bv, Wp, bp, _trace=False, _trace_kwargs=None):
    x, Wk, bk, Wq, bq, Wv, bv, Wp, bp = [
        np.asarray(a) for a in (x, Wk, bk, Wq, bq, Wv, bv, Wp, bp)]
    if "nc" not in _cache:
        _cache["nc"] = _build()
    nc = _cache["nc"]
    in_maps = _shard_inputs(x, Wk, bk, Wq, bq, Wv, bv, Wp, bp)
    kw = dict(_trace_kwargs or {})
    res = run_bass_kernel_spmd(nc, in_maps, core_ids=list(range(NCORES)),
                               trace=_trace, **kw)
    out = np.empty((B, T, C), np.float32)
    for b in range(B):
        yp = res.results[2 * b]["ypT"] + res.results[2 * b + 1]["ypT"]
        out[b] = yp.T + bp[None, :]
    if _trace:
        _cache["last_results"] = res
    return out
